# revision 11
# baseline (speedup 1.0000x reference)
"""BiGRU (S=512, B=64, I=256, H=512, L=2) Trainium2 Bass kernel.

Strategy: 4-way batch split x 2-way direction split across 8 NeuronCores.
Cores 0-3 run the forward GRU chain (layers 0 and 1) for batch quarters
0-3; cores 4-7 run the backward chain (fed time-reversed input, so the
device program is identical on every core).  Per layer each core does:

  P-phase: gxT = Wih @ xT + bias  (big efficient matmul, bf16, weights
           stationary, all timesteps as the moving operand), written to
           DRAM in a scan-friendly blocked layout.
  S-phase: 512-step sequential GRU scan.  Per step the 48 Whh weight
           tiles stream through the PE; the moving operand is the PAIR
           (m, p1) with h = m + p1 (linearity of the recurrent matmul),
           so the final h combine is off the critical path (done lazily
           on GpSimd one step later).  Gate math chain:
             z  = sigmoid(ghz)         omz = sigmoid(-ghz)
             p1 = z * h_prev           r   = sigmoid(ghr)
             t  = tanh(r*ghn + gxn)  (split in two fold-halves)
             m  = omz * t
           The n-gate PSUM is split in two half tiles so the tanh chain
           starts as soon as the first half's matmuls land.

Between layers the forward/backward partners exchange their hidden-state
sequences with pairwise AllGathers split into 4 time-chunks, each fired
as soon as its chunk of y0ex is written during the scan (overlapping the
collective with the scan).  Final un-transpose / un-reverse of the
output happens on the host.
"""

import os
import sys
import numpy as np

for _p in ("/opt/trn_rl_repo", "/root/.axon_site/_ro/trn_rl_repo"):
    if os.path.isdir(_p) and _p not in sys.path:
        sys.path.insert(0, _p)

import ml_dtypes
from contextlib import ExitStack

import concourse.bass as bass
import concourse.tile as tile
from concourse import bacc, mybir
from concourse.bass import ts
from concourse.bass_utils import run_bass_kernel_spmd

BF16 = mybir.dt.bfloat16
F32 = mybir.dt.float32
AF = mybir.ActivationFunctionType
ALU = mybir.AluOpType

S, B, I, H, L = 512, 64, 256, 512, 2
G = 3 * H            # 1536 gate rows (r, z, n)
NCORE = 8
BQ = B // 4          # 16 batch per core
SB = S * BQ          # 8192 moving columns
F = H // 128         # 4 h-fold chunks
M12 = G // 128       # 12 gate chunks
KI0 = I // 128       # 2 contraction chunks, layer-0 input proj
KI1 = 2 * H // 128   # 8 contraction chunks, layer-1 input proj
NCOL = 512           # P-phase moving chunk width
TBLK = 16            # gx prefetch / y writeback block (steps)
NB = S // TBLK       # 32 blocks
NCH = 8              # AllGather chunks
NBC = NB // NCH      # blocks per chunk
SBC = SB // NCH      # columns per chunk
UB = NCOL // BQ      # steps covered by one P-phase column chunk (32)


def _p_phase(ctx, tc, nc, wT_dram, gbias_dram, gx_dram, ki, rhs_fn, tag,
             c_order=None):
    """out = W @ xT + bias, bf16, written to gx_dram in blocked layout
    [128, NB, M12, TBLK*BQ]."""
    nc_ = nc
    wpool = ctx.enter_context(tc.tile_pool(name=f"w_{tag}", bufs=1))
    bpool = ctx.enter_context(tc.tile_pool(name=f"b_{tag}", bufs=1))
    psum = ctx.enter_context(tc.tile_pool(name=f"ps_{tag}", bufs=4, space="PSUM"))
    stg = ctx.enter_context(tc.tile_pool(name=f"st_{tag}", bufs=4))

    wsb = wpool.tile([128, ki, G], BF16)
    nc_.sync.dma_start(wsb[:], wT_dram.ap().rearrange("(k p) g -> p k g", p=128))
    gb = bpool.tile([128, M12], F32)
    nc_.sync.dma_start(gb[:], gbias_dram.ap())

    # gx blocked layout: [128, NB, M12, TBLK*BQ]
    gx_r = gx_dram.ap().rearrange("p (blk m c) -> p blk m c", m=M12, c=TBLK * BQ)
    nub = NCOL // (TBLK * BQ)  # u-blocks per column chunk (2)
    if c_order is None:
        c_order = range(SB // NCOL)
    for c in c_order:
        rhs_tiles = rhs_fn(c)  # list of ki APs, each [128, NCOL] bf16
        for m in range(M12):
            ps = psum.tile([128, NCOL], F32)
            for k in range(ki):
                nc_.tensor.matmul(
                    ps[:],
                    lhsT=wsb[:, k, ts(m, 128)],
                    rhs=rhs_tiles[k],
                    start=(k == 0),
                    stop=(k == ki - 1),
                )
            out = stg.tile([128, NCOL], BF16)
            if m % 2 == 0:
                nc_.scalar.activation(out[:], ps[:], AF.Identity, bias=gb[:, m : m + 1])
            else:
                nc_.vector.tensor_scalar_add(out[:], ps[:], gb[:, m : m + 1])
            nc_.sync.dma_start(
                gx_r[:, ts(c, nub), m, :],
                out[:].rearrange("p (i c) -> p i c", c=TBLK * BQ),
            )


def _s_phase(ctx, tc, nc, whhT_dram, nbias_dram, gx_dram, layer, y0own,
             y1T_dram, ident_dram, y0ex_chunks, ag_fn=None):
    """512-step GRU scan with linearity-split moving operand (m, p1).

    ag_fn(chunk) is called right after the last y0ex block of `chunk` is
    written, so the pairwise AllGather for that chunk overlaps the scan."""
    nc_ = nc
    tag = f"s{layer}"
    wpool = ctx.enter_context(tc.tile_pool(name=f"whh_{tag}", bufs=1))
    cpool = ctx.enter_context(tc.tile_pool(name=f"c_{tag}", bufs=1))
    gxp = ctx.enter_context(tc.tile_pool(name=f"gx_{tag}", bufs=2))
    psum = ctx.enter_context(tc.tile_pool(name=f"ps_{tag}", bufs=2, space="PSUM"))
    gp = ctx.enter_context(tc.tile_pool(name=f"g_{tag}", bufs=2))
    mp = ctx.enter_context(tc.tile_pool(name=f"mp_{tag}", bufs=3))
    yp = ctx.enter_context(tc.tile_pool(name=f"y_{tag}", bufs=2))

    whh = wpool.tile([128, F, G], BF16)
    nc_.sync.dma_start(whh[:], whhT_dram.ap().rearrange("(k p) g -> p k g", p=128))
    ident = cpool.tile([128, 128], BF16)
    nc_.sync.dma_start(ident[:], ident_dram.ap())
    # nbias comes pre-broadcast from the host as bf16 [128, F*BQ]
    nbx = cpool.tile([128, F, BQ], BF16)
    nc_.sync.dma_start(nbx[:], nbias_dram.ap().rearrange("p (f b) -> p f b", b=BQ))
    zero_bf = cpool.tile([128, F, BQ], BF16)
    nc_.vector.memset(zero_bf[:], 0.0)

    gx_r = gx_dram.ap().rearrange("p (blk m c) -> p blk m c", m=M12, c=TBLK * BQ)
    y1_r = None
    if y1T_dram is not None:
        y1_r = y1T_dram.ap().rearrange("(f p) c -> p f c", p=128)

    def load_block(blk):
        t = gxp.tile([128, M12, TBLK * BQ], BF16)
        nc_.sync.dma_start(t[:], gx_r[:, blk, :, :])
        return t

    def write_block(wb, y1sb):
        if layer == 0:
            chunk = ((S - 1 - wb * TBLK) * BQ) // SBC
            y0e = y0ex_chunks[chunk].ap()
            lo = (S - 1 - wb * TBLK) * BQ - chunk * SBC
            for f in range(F):
                dst = bass.AP(
                    tensor=y0e.tensor,
                    offset=f * 128 * SBC + lo,
                    ap=[[SBC, 128], [-BQ, TBLK], [1, BQ]],
                )
                src = y0own[:, f, ts(wb, TBLK * BQ)].rearrange(
                    "p (t b) -> p t b", b=BQ)
                nc_.sync.dma_start(dst, src)
        else:
            nc_.sync.dma_start(y1_r[:, :, ts(wb, TBLK * BQ)], y1sb[:])

    # scan state
    m_prev = zero_bf[:]
    p1_prev = zero_bf[:]
    hm1 = zero_bf[:]
    gxb_cur = load_block(0)
    gxb_next = None
    y1sb_cur = None
    y1sb_prev = None

    # tile orders inside the PE burst
    zr_order = [(f, k) for f in range(F) for k in (0, 1)] + \
               [(f, k) for f in range(F) for k in (2, 3)]
    n_orders = [[(f, k) for f in half for k in range(F)]
                for half in ((0, 1), (2, 3))]

    for u in range(S):
        blk, j = divmod(u, TBLK)
        if j == 0:
            if u > 0:
                gxb_cur = gxb_next
            if blk + 1 < NB:
                gxb_next = load_block(blk + 1)
            if layer == 1:
                y1sb_prev = y1sb_cur
                y1sb_cur = yp.tile([128, F, TBLK * BQ], BF16, tag="y1sb")

        # ---- lazy h_{u-1} = m + p1 (off the critical path) ----
        if u >= 1:
            up = u - 1
            bp, jp = divmod(up, TBLK)
            if layer == 0:
                hslot = y0own[:, :, ts(up, BQ)]
            else:
                ysb = y1sb_cur if bp == blk else y1sb_prev
                hslot = ysb[:, :, ts(jp, BQ)]
            nc_.vector.tensor_tensor(hslot, m_prev, p1_prev, ALU.add)
            hm1 = hslot
        if j == 0 and u > 0:
            wb = blk - 1
            write_block(wb, y1sb_prev)
            if ag_fn is not None and (wb + 1) % NBC == 0:
                ag_fn(((S - 1 - wb * TBLK) * BQ) // SBC)

        # ---- PE burst: gh = Whh @ (m + p1) + inj, grouped z, r, n0, n1 ----
        psz = psum.tile([128, F, BQ], F32, tag="z")
        psr = psum.tile([128, F, BQ], F32, tag="r")
        psn0 = psum.tile([128, 2, BQ], F32, tag="n0")
        psn1 = psum.tile([128, 2, BQ], F32, tag="n1")

        for gate, ps, order, minj in (
            ("z", psz, zr_order, None),
            ("r", psr, zr_order, None),
            ("n0", psn0, n_orders[0], 0),
            ("n1", psn1, n_orders[1], 2),
        ):
            if gate == "z":
                inj = gxb_cur[:, F : 2 * F, ts(j, BQ)]
                m0 = F
            elif gate == "r":
                inj = gxb_cur[:, 0:F, ts(j, BQ)]
                m0 = 0
            else:
                inj = nbx[:, ts(minj // 2, 2), :]
                m0 = 2 * F
            nc_.tensor.matmul(ps[:], lhsT=ident[:], rhs=inj,
                              start=True, stop=False, skip_group_check=True)
            last = order[-1]
            for (f, k) in order:
                m = m0 + f
                pslot = ps[:, f - minj, :] if gate.startswith("n") else ps[:, f, :]
                w = whh[:, k, ts(m, 128)]
                nc_.tensor.matmul(pslot, lhsT=w, rhs=m_prev[:, k, :],
                                  start=False, stop=False, skip_group_check=True)
                nc_.tensor.matmul(pslot, lhsT=w, rhs=p1_prev[:, k, :],
                                  start=False,
                                  stop=((f, k) == last),
                                  skip_group_check=True)

        # ---- gate math ----
        z = gp.tile([128, F, BQ], F32, tag="z")
        nc_.scalar.activation(z[:], psz[:], AF.Sigmoid)
        p1 = mp.tile([128, F, BQ], BF16, tag="p1")
        nc_.vector.tensor_tensor(p1[:], z[:], hm1, ALU.mult)
        r = gp.tile([128, F, BQ], F32, tag="r")
        nc_.scalar.activation(r[:], psr[:], AF.Sigmoid)
        omz = gp.tile([128, F, BQ], F32, tag="omz")
        nc_.scalar.activation(omz[:], psz[:], AF.Sigmoid, scale=-1.0)
        mnew = mp.tile([128, F, BQ], BF16, tag="m")
        t2 = gp.tile([128, F, BQ], F32, tag="t2")
        n = gp.tile([128, F, BQ], F32, tag="n")
        for hh, psn in ((0, psn0), (1, psn1)):
            sl = ts(hh, 2)
            t1 = gp.tile([128, 2, BQ], F32, tag=f"t1{hh}")
            nc_.vector.tensor_tensor(t1[:], r[:, sl, :], psn[:], ALU.mult)
            nc_.vector.tensor_tensor(t2[:, sl, :], t1[:],
                                     gxb_cur[:, 2 * F + 2 * hh: 2 * F + 2 * hh + 2,
                                             ts(j, BQ)],
                                     ALU.add)
            nc_.scalar.activation(n[:, sl, :], t2[:, sl, :], AF.Tanh)
            nc_.vector.tensor_tensor(mnew[:, sl, :], omz[:, sl, :], n[:, sl, :],
                                     ALU.mult)

        m_prev = mnew[:]
        p1_prev = p1[:]

    # final h materialization + last block writeback
    if layer == 0:
        hslot = y0own[:, :, ts(S - 1, BQ)]
    else:
        hslot = y1sb_cur[:, :, ts(TBLK - 1, BQ)]
    nc_.vector.tensor_tensor(hslot, m_prev, p1_prev, ALU.add)
    write_block(NB - 1, y1sb_cur)
    if ag_fn is not None:
        ag_fn(0)


def build_program(debug=False):
    nc = bacc.Bacc("TRN2", target_bir_lowering=False, debug=debug,
                   num_devices=NCORE)

    def din(name, shape, dt):
        return nc.dram_tensor(name, list(shape), dt, kind="ExternalInput")

    xT = din("xT", (I, SB), BF16)
    wih0T = din("wih0T", (I, G), BF16)
    whh0T = din("whh0T", (H, G), BF16)
    wih1T = din("wih1T", (2 * H, G), BF16)
    whh1T = din("whh1T", (H, G), BF16)
    gbias0 = din("gbias0", (128, M12), F32)
    gbias1 = din("gbias1", (128, M12), F32)
    nbias0 = din("nbias0", (128, F * BQ), BF16)
    nbias1 = din("nbias1", (128, F * BQ), BF16)
    ident = din("ident", (128, 128), BF16)

    y1T = nc.dram_tensor("y1T", [H, SB], BF16, kind="ExternalOutput")

    gx0T = nc.dram_tensor("gx0T", [128, NB * M12 * TBLK * BQ], BF16)
    gx1T = nc.dram_tensor("gx1T", [128, NB * M12 * TBLK * BQ], BF16)
    y0ex_chunks = [nc.dram_tensor(f"y0ex{c}", [H, SBC], BF16)
                   for c in range(NCH)]
    y0g_chunks = [nc.dram_tensor(f"y0g{c}", [2, H, SBC], BF16)
                  for c in range(NCH)]
    y0loc_chunks = [nc.dram_tensor(f"y0loc{c}", [H, SBC], BF16)
                    for c in range(NCH)]

    groups = [[2 * q, 2 * q + 1] for q in range(4)]

    with tile.TileContext(nc) as tc:
        with ExitStack() as ctx:
            # ---- P0: layer-0 input projection ----
            with ExitStack() as pctx:
                xpool = pctx.enter_context(tc.tile_pool(name="xsb", bufs=1))
                xsb = xpool.tile([128, KI0, SB], BF16)
                nc.sync.dma_start(xsb[:], xT.ap().rearrange("(k p) c -> p k c", p=128))
                _p_phase(pctx, tc, nc, wih0T, gbias0, gx0T, KI0,
                         lambda c: [xsb[:, k, ts(c, NCOL)] for k in range(KI0)], "p0")

            # ---- S0: layer-0 scan; y0own holds the h sequence in SBUF.
            # Pairwise AllGathers fire per chunk as y0ex streams out. ----
            rank = nc.gpsimd.cc_rank(groups)

            def ag_fn(c):
                nc.gpsimd.collective_compute(
                    "AllGather", ALU.bypass,
                    ins=[y0ex_chunks[c].ap()], outs=[y0g_chunks[c].ap()],
                    replica_groups=groups,
                )
                with tc.If(rank < 1) as cmp:
                    for rr in range(4):
                        nc.gpsimd.dma_start(
                            y0loc_chunks[c].ap()[ts(rr, 128), :],
                            y0g_chunks[c].ap()[1, ts(rr, 128), :])
                with cmp.Else():
                    for rr in range(4):
                        nc.gpsimd.dma_start(
                            y0loc_chunks[c].ap()[ts(rr, 128), :],
                            y0g_chunks[c].ap()[0, ts(rr, 128), :])

            y0pool = ctx.enter_context(tc.tile_pool(name="y0own", bufs=1))
            y0own = y0pool.tile([128, F, SB], BF16)
            with ExitStack() as sctx:
                _s_phase(sctx, tc, nc, whh0T, nbias0, gx0T, 0, y0own, None,
                         ident, y0ex_chunks, ag_fn=ag_fn)

            # ---- P1: layer-1 input projection (descending c: AG overlap) ----
            with ExitStack() as pctx:
                ppool = pctx.enter_context(tc.tile_pool(name="part", bufs=3))
                y0l_r = [t.ap().rearrange("(k p) c -> p k c", p=128)
                         for t in y0loc_chunks]
                cpc = SBC // NCOL  # NCOL chunks per AG chunk

                def rhs1(c):
                    part = ppool.tile([128, F, NCOL], BF16)
                    ch, off = divmod(c, cpc)
                    nc.sync.dma_start(part[:], y0l_r[ch][:, :, ts(off, NCOL)])
                    return [y0own[:, k, ts(c, NCOL)] for k in range(F)] + \
                           [part[:, k, :] for k in range(F)]

                _p_phase(pctx, tc, nc, wih1T, gbias1, gx1T, KI1, rhs1, "p1",
                         c_order=list(reversed(range(SB // NCOL))))

            # ---- S1: layer-1 scan -> y1T ----
            with ExitStack() as sctx:
                _s_phase(sctx, tc, nc, whh1T, nbias1, gx1T, 1, None, y1T,
                         ident, None)

    nc.compile()
    return nc


_PROGRAM_CACHE = {}


def _get_program():
    if "nc" not in _PROGRAM_CACHE:
        _PROGRAM_CACHE["nc"] = build_program()
    return _PROGRAM_CACHE["nc"]


def _host_inputs(inputs):
    """Build the 8 per-core input maps from the full problem inputs."""
    bf = ml_dtypes.bfloat16
    x = np.asarray(inputs["input"], np.float32)            # (S, B, I)
    in_maps = []
    for c in range(NCORE):
        fwd = c % 2 == 0
        q = c // 2
        d = "f" if fwd else "b"
        xq = x[:, q * BQ:(q + 1) * BQ, :]
        if not fwd:
            xq = xq[::-1]
        xTv = np.ascontiguousarray(xq.transpose(2, 0, 1).reshape(I, SB))

        def wT(wname):
            return np.ascontiguousarray(np.asarray(inputs[wname], np.float32).T)

        wih0 = wT(f"Wih_{d}0")        # (I, G)
        whh0 = wT(f"Whh_{d}0")        # (H, G)
        wih1_full = wT(f"Wih_{d}1")   # (2H, G); rows = y0 features [hf | hb]
        own_sl = slice(0, H) if fwd else slice(H, 2 * H)
        par_sl = slice(H, 2 * H) if fwd else slice(0, H)
        wih1 = np.concatenate([wih1_full[own_sl], wih1_full[par_sl]], axis=0)
        whh1 = wT(f"Whh_{d}1")

        def gbias(layer):
            bih = np.asarray(inputs[f"bih_{d}{layer}"], np.float32)
            bhh = np.asarray(inputs[f"bhh_{d}{layer}"], np.float32)
            gb = np.concatenate([bih[:2 * H] + bhh[:2 * H], bih[2 * H:]])
            return np.ascontiguousarray(gb.reshape(M12, 128).T)  # [128, M12]

        def nbias(layer):
            bhh = np.asarray(inputs[f"bhh_{d}{layer}"], np.float32)
            nb = bhh[2 * H:].reshape(F, 128).T  # [128, F]
            return np.ascontiguousarray(
                np.broadcast_to(nb[:, :, None], (128, F, BQ)).reshape(
                    128, F * BQ)).astype(bf)

        in_maps.append({
            "xT": xTv.astype(bf),
            "wih0T": wih0.astype(bf), "whh0T": whh0.astype(bf),
            "wih1T": wih1.astype(bf), "whh1T": whh1.astype(bf),
            "gbias0": gbias(0), "gbias1": gbias(1),
            "nbias0": nbias(0), "nbias1": nbias(1),
            "ident": np.eye(128, dtype=bf),
        })
    return in_maps


def kernel(**inputs) -> np.ndarray:
    nc = _get_program()
    in_maps = _host_inputs(inputs)
    trace = bool(int(os.environ.get("BIGRU_TRACE", "0")))
    kw = {}
    if trace and os.environ.get("BIGRU_TRACE_DIR"):
        kw["tmpdir"] = os.environ["BIGRU_TRACE_DIR"]
    res = run_bass_kernel_spmd(nc, in_maps, list(range(NCORE)), trace=trace, **kw)
    if trace and res.exec_time_ns is not None:
        print(f"HW exec time: {res.exec_time_ns} ns")
        _PROGRAM_CACHE["exec_time_ns"] = res.exec_time_ns
        _PROGRAM_CACHE["profile_json"] = res.profile_json

    out = np.empty((S, B, 2 * H), np.float32)
    for c in range(NCORE):
        fwd = c % 2 == 0
        q = c // 2
        y = np.asarray(res.results[c]["y1T"], dtype=np.float32)
        y = y.reshape(H, S, BQ).transpose(1, 2, 0)  # (S, BQ, H)
        if not fwd:
            y = y[::-1]
        out[:, q * BQ:(q + 1) * BQ, (0 if fwd else H):(H if fwd else 2 * H)] = y
    return out


# revision 19
# speedup vs baseline: 1.2657x; 1.2657x over previous
"""BiGRU (S=512, B=64, I=256, H=512, L=2) Trainium2 Bass kernel.

Strategy: 4-way batch split x 2-way direction split across 8 NeuronCores.
Cores 0-3 run the forward GRU chain (layers 0 and 1) for batch quarters
0-3; cores 4-7 run the backward chain (fed time-reversed input, so the
device program is identical on every core).  Per layer each core does:

  P-phase: gxT = Wih @ xT + bias  (big efficient matmul, bf16, weights
           stationary, all timesteps as the moving operand), written to
           DRAM in a scan-friendly blocked layout.
  S-phase: 512-step sequential GRU scan.  Per step the 48 Whh weight
           tiles stream through the PE; the moving operand is the PAIR
           (m, p1) with h = m + p1 (linearity of the recurrent matmul),
           so the final h combine is off the critical path (done lazily
           on GpSimd one step later).  Gate math chain:
             z  = sigmoid(ghz)         omz = sigmoid(-ghz)
             p1 = z * h_prev           r   = sigmoid(ghr)
             t  = tanh(r*ghn + gxn)  (split in two fold-halves)
             m  = omz * t
           The n-gate PSUM is split in two half tiles so the tanh chain
           starts as soon as the first half's matmuls land.

Between layers the forward/backward partners exchange their hidden-state
sequences with pairwise AllGathers split into 4 time-chunks, each fired
as soon as its chunk of y0ex is written during the scan (overlapping the
collective with the scan).  Final un-transpose / un-reverse of the
output happens on the host.
"""

import os
import sys
import numpy as np

for _p in ("/opt/trn_rl_repo", "/root/.axon_site/_ro/trn_rl_repo"):
    if os.path.isdir(_p) and _p not in sys.path:
        sys.path.insert(0, _p)

import ml_dtypes
from contextlib import ExitStack

import concourse.bass as bass
import concourse.tile as tile
from concourse import bacc, mybir
from concourse.bass import ts
from concourse.bass_utils import run_bass_kernel_spmd

BF16 = mybir.dt.bfloat16
FP8 = mybir.dt.float8e4
F32 = mybir.dt.float32
AF = mybir.ActivationFunctionType
ALU = mybir.AluOpType

# Whh is stored in fp8-e4m3 scaled so max|W| -> 240; the descale folds into
# the activation `scale` operand (gates) / one fused scalar_tensor_tensor (n).
WSCALE = float(240.0 * np.sqrt(512.0))
SINV = float(1.0 / WSCALE)

S, B, I, H, L = 512, 64, 256, 512, 2
G = 3 * H            # 1536 gate rows (r, z, n)
NCORE = 8
BQ = B // 4          # 16 batch per core
SB = S * BQ          # 8192 moving columns
F = H // 128         # 4 h-fold chunks
M12 = G // 128       # 12 gate chunks
KI0 = I // 128       # 2 contraction chunks, layer-0 input proj
KI1 = 2 * H // 128   # 8 contraction chunks, layer-1 input proj
NCOL = 512           # P-phase moving chunk width
TBLK = 16            # gx prefetch / y writeback block (steps)
NB = S // TBLK       # 32 blocks
NCH = 8              # AllGather chunks
NBC = NB // NCH      # blocks per chunk
SBC = SB // NCH      # columns per chunk
UB = NCOL // BQ      # steps covered by one P-phase column chunk (32)


def _p_phase(ctx, tc, nc, wT_dram, gbias_dram, gx_dram, ki, rhs_fn, tag,
             c_order=None):
    """out = W @ xT + bias, bf16, written to gx_dram in blocked layout
    [128, NB, M12, TBLK*BQ]."""
    nc_ = nc
    wpool = ctx.enter_context(tc.tile_pool(name=f"w_{tag}", bufs=1))
    bpool = ctx.enter_context(tc.tile_pool(name=f"b_{tag}", bufs=1))
    psum = ctx.enter_context(tc.tile_pool(name=f"ps_{tag}", bufs=4, space="PSUM"))
    stg = ctx.enter_context(tc.tile_pool(name=f"st_{tag}", bufs=4))

    wsb = wpool.tile([128, ki, G], BF16)
    nc_.sync.dma_start(wsb[:], wT_dram.ap().rearrange("(k p) g -> p k g", p=128))
    gb = bpool.tile([128, M12], F32)
    nc_.sync.dma_start(gb[:], gbias_dram.ap())

    # gx blocked layout: [128, NB, M12, TBLK*BQ]
    gx_r = gx_dram.ap().rearrange("p (blk m c) -> p blk m c", m=M12, c=TBLK * BQ)
    nub = NCOL // (TBLK * BQ)  # u-blocks per column chunk (2)
    if c_order is None:
        c_order = range(SB // NCOL)
    for c in c_order:
        rhs_tiles = rhs_fn(c)  # list of ki APs, each [128, NCOL] bf16
        for m in range(M12):
            ps = psum.tile([128, NCOL], F32)
            for k in range(ki):
                nc_.tensor.matmul(
                    ps[:],
                    lhsT=wsb[:, k, ts(m, 128)],
                    rhs=rhs_tiles[k],
                    start=(k == 0),
                    stop=(k == ki - 1),
                )
            # r,z chunks (m<8) are pre-scaled by WSCALE so the S-phase can
            # descale the whole PSUM (Whh fp8 part + injected gx) at once.
            # gbias for m<8 comes pre-scaled from the host.
            out = stg.tile([128, NCOL], BF16)
            sc = WSCALE if m < 2 * F else 1.0
            if m % 2 == 0:
                nc_.scalar.activation(out[:], ps[:], AF.Identity,
                                      bias=gb[:, m : m + 1], scale=sc)
            else:
                nc_.vector.tensor_scalar(out[:], ps[:], sc, gb[:, m : m + 1],
                                         ALU.mult, ALU.add)
            nc_.sync.dma_start(
                gx_r[:, ts(c, nub), m, :],
                out[:].rearrange("p (i c) -> p i c", c=TBLK * BQ),
            )


def _s_phase(ctx, tc, nc, whhT_dram, nbias_dram, gx_dram, layer, y0own,
             y1T_dram, ident_dram, y0ex_chunks, ag_fn=None):
    """512-step GRU scan with linearity-split moving operand (m, p1).

    ag_fn(chunk) is called right after the last y0ex block of `chunk` is
    written, so the pairwise AllGather for that chunk overlaps the scan."""
    nc_ = nc
    tag = f"s{layer}"
    wpool = ctx.enter_context(tc.tile_pool(name=f"whh_{tag}", bufs=1))
    cpool = ctx.enter_context(tc.tile_pool(name=f"c_{tag}", bufs=1))
    gxp = ctx.enter_context(tc.tile_pool(name=f"gx_{tag}", bufs=2))
    psum = ctx.enter_context(tc.tile_pool(name=f"ps_{tag}", bufs=2, space="PSUM"))
    gp = ctx.enter_context(tc.tile_pool(name=f"g_{tag}", bufs=2))
    yp = ctx.enter_context(tc.tile_pool(name=f"y_{tag}", bufs=2))

    whh = wpool.tile([128, F, G], FP8)
    nc_.sync.dma_start(whh[:], whhT_dram.ap().rearrange("(k p) g -> p k g", p=128))
    ident = cpool.tile([128, 128], BF16)
    nc_.sync.dma_start(ident[:], ident_dram.ap())
    # nbias comes pre-broadcast (and pre-scaled by WSCALE) from the host
    nbx = cpool.tile([128, F, BQ], BF16)
    nc_.sync.dma_start(nbx[:], nbias_dram.ap().rearrange("p (f b) -> p f b", b=BQ))
    zero_bf = cpool.tile([128, F, BQ], BF16)
    nc_.vector.memset(zero_bf[:], 0.0)

    gx_r = gx_dram.ap().rearrange("p (blk m c) -> p blk m c", m=M12, c=TBLK * BQ)
    y1_r = None
    if y1T_dram is not None:
        y1_r = y1T_dram.ap().rearrange("(f p) c -> p f c", p=128)

    def load_block(blk):
        t = gxp.tile([128, M12, TBLK * BQ], BF16)
        nc_.sync.dma_start(t[:], gx_r[:, blk, :, :])
        return t

    def write_block(wb, y1sb):
        if layer == 0:
            chunk = ((S - 1 - wb * TBLK) * BQ) // SBC
            y0e = y0ex_chunks[chunk].ap()
            lo = (S - 1 - wb * TBLK) * BQ - chunk * SBC
            for f in range(F):
                dst = bass.AP(
                    tensor=y0e.tensor,
                    offset=f * 128 * SBC + lo,
                    ap=[[SBC, 128], [-BQ, TBLK], [1, BQ]],
                )
                src = y0own[:, f, ts(wb, TBLK * BQ)].rearrange(
                    "p (t b) -> p t b", b=BQ)
                nc_.sync.dma_start(dst, src)
        else:
            nc_.sync.dma_start(y1_r[:, :, ts(wb, TBLK * BQ)], y1sb[:])

    # scan state
    hm1 = zero_bf[:]
    gxb_cur = load_block(0)
    gxb_next = None
    y1sb_cur = None

    # tile orders inside the PE burst: r,z consume h halves in order so the
    # next step can start as soon as the low half of h lands; the n group is
    # plain fold-major.
    zr_order = [(f, k) for f in range(F) for k in (0, 1)] + \
               [(f, k) for f in range(F) for k in (2, 3)]
    n_order = [(f, k) for f in range(F) for k in range(F)]

    for u in range(S):
        blk, j = divmod(u, TBLK)
        if j == 0:
            if u > 0:
                gxb_cur = gxb_next
            if blk + 1 < NB:
                gxb_next = load_block(blk + 1)
            if layer == 1:
                y1sb_cur = yp.tile([128, F, TBLK * BQ], BF16, tag="y1sb")

        # ---- PE burst, gate group order r, z, n ----
        psr = psum.tile([128, F, BQ], F32, tag="r")
        psz = psum.tile([128, F, BQ], F32, tag="z")
        psn = psum.tile([128, F, BQ], F32, tag="n")

        for gate, ps, order, inj, m0 in (
            ("r", psr, zr_order, gxb_cur[:, 0:F, ts(j, BQ)], 0),
            ("z", psz, zr_order, gxb_cur[:, F: 2 * F, ts(j, BQ)], F),
            ("n", psn, n_order, nbx[:], 2 * F),
        ):
            nc_.tensor.matmul(ps[:], lhsT=ident[:], rhs=inj,
                              start=True, stop=False, skip_group_check=True)
            last = order[-1]
            for (f, k) in order:
                nc_.tensor.matmul(ps[:, f, :],
                                  lhsT=whh[:, k, ts(m0 + f, 128)],
                                  rhs=hm1[:, k, :],
                                  start=False, stop=((f, k) == last),
                                  skip_group_check=True)

        # ---- gate math; critical chain: t1 -> t2 -> tanh -> m -> h ----
        r = gp.tile([128, F, BQ], F32, tag="r")
        nc_.scalar.activation(r[:], psr[:], AF.Sigmoid, scale=SINV)
        z = gp.tile([128, F, BQ], F32, tag="z")
        nc_.scalar.activation(z[:], psz[:], AF.Sigmoid, scale=SINV)
        omz = gp.tile([128, F, BQ], F32, tag="omz")
        nc_.scalar.activation(omz[:], psz[:], AF.Sigmoid, scale=-SINV)

        t1 = gp.tile([128, F, BQ], F32, tag="t1")
        nc_.vector.scalar_tensor_tensor(t1[:], psn[:], SINV, r[:],
                                        ALU.mult, ALU.mult)
        t2 = gp.tile([128, F, BQ], F32, tag="t2")
        nc_.vector.tensor_tensor(t2[:], t1[:],
                                 gxb_cur[:, 2 * F: 3 * F, ts(j, BQ)], ALU.add)
        n = gp.tile([128, F, BQ], F32, tag="n")
        nc_.scalar.activation(n[:], t2[:], AF.Tanh)

        p1 = gp.tile([128, F, BQ], F32, tag="p1")
        nc_.vector.tensor_tensor(p1[:], z[:], hm1, ALU.mult)
        m = gp.tile([128, F, BQ], F32, tag="m")
        nc_.vector.tensor_tensor(m[:], omz[:], n[:], ALU.mult)

        if layer == 0:
            hslot = y0own[:, :, ts(u, BQ)]
        else:
            hslot = y1sb_cur[:, :, ts(j, BQ)]
        # h = m + p1, low half first so the next burst can start early
        nc_.vector.tensor_tensor(hslot[:, 0:2, :], m[:, 0:2, :],
                                 p1[:, 0:2, :], ALU.add)
        nc_.vector.tensor_tensor(hslot[:, 2:4, :], m[:, 2:4, :],
                                 p1[:, 2:4, :], ALU.add)
        hm1 = hslot

        if j == TBLK - 1:
            write_block(blk, y1sb_cur)
            if ag_fn is not None and (blk + 1) % NBC == 0:
                ag_fn(((S - 1 - blk * TBLK) * BQ) // SBC)


def build_program(debug=False):
    nc = bacc.Bacc("TRN2", target_bir_lowering=False, debug=debug,
                   num_devices=NCORE)

    def din(name, shape, dt):
        return nc.dram_tensor(name, list(shape), dt, kind="ExternalInput")

    xT = din("xT", (I, SB), BF16)
    wih0T = din("wih0T", (I, G), BF16)
    whh0T = din("whh0T", (H, G), FP8)
    wih1T = din("wih1T", (2 * H, G), BF16)
    whh1T = din("whh1T", (H, G), FP8)
    gbias0 = din("gbias0", (128, M12), F32)
    gbias1 = din("gbias1", (128, M12), F32)
    nbias0 = din("nbias0", (128, F * BQ), BF16)
    nbias1 = din("nbias1", (128, F * BQ), BF16)
    ident = din("ident", (128, 128), BF16)

    y1T = nc.dram_tensor("y1T", [H, SB], BF16, kind="ExternalOutput")

    gx0T = nc.dram_tensor("gx0T", [128, NB * M12 * TBLK * BQ], BF16)
    gx1T = nc.dram_tensor("gx1T", [128, NB * M12 * TBLK * BQ], BF16)
    y0ex_chunks = [nc.dram_tensor(f"y0ex{c}", [H, SBC], BF16)
                   for c in range(NCH)]
    y0g_chunks = [nc.dram_tensor(f"y0g{c}", [2, H, SBC], BF16)
                  for c in range(NCH)]
    y0loc_chunks = [nc.dram_tensor(f"y0loc{c}", [H, SBC], BF16)
                    for c in range(NCH)]

    groups = [[2 * q, 2 * q + 1] for q in range(4)]

    with tile.TileContext(nc) as tc:
        with ExitStack() as ctx:
            # ---- P0: layer-0 input projection ----
            with ExitStack() as pctx:
                xpool = pctx.enter_context(tc.tile_pool(name="xsb", bufs=1))
                xsb = xpool.tile([128, KI0, SB], BF16)
                nc.sync.dma_start(xsb[:], xT.ap().rearrange("(k p) c -> p k c", p=128))
                _p_phase(pctx, tc, nc, wih0T, gbias0, gx0T, KI0,
                         lambda c: [xsb[:, k, ts(c, NCOL)] for k in range(KI0)], "p0")

            # ---- S0: layer-0 scan; y0own holds the h sequence in SBUF.
            # Pairwise AllGathers fire per chunk as y0ex streams out. ----
            rank = nc.gpsimd.cc_rank(groups)

            def ag_fn(c):
                nc.gpsimd.collective_compute(
                    "AllGather", ALU.bypass,
                    ins=[y0ex_chunks[c].ap()], outs=[y0g_chunks[c].ap()],
                    replica_groups=groups,
                )
                with tc.If(rank < 1) as cmp:
                    for rr in range(4):
                        nc.gpsimd.dma_start(
                            y0loc_chunks[c].ap()[ts(rr, 128), :],
                            y0g_chunks[c].ap()[1, ts(rr, 128), :])
                with cmp.Else():
                    for rr in range(4):
                        nc.gpsimd.dma_start(
                            y0loc_chunks[c].ap()[ts(rr, 128), :],
                            y0g_chunks[c].ap()[0, ts(rr, 128), :])

            y0pool = ctx.enter_context(tc.tile_pool(name="y0own", bufs=1))
            y0own = y0pool.tile([128, F, SB], BF16)
            with ExitStack() as sctx:
                _s_phase(sctx, tc, nc, whh0T, nbias0, gx0T, 0, y0own, None,
                         ident, y0ex_chunks, ag_fn=ag_fn)

            # ---- P1: layer-1 input projection (descending c: AG overlap) ----
            with ExitStack() as pctx:
                ppool = pctx.enter_context(tc.tile_pool(name="part", bufs=3))
                y0l_r = [t.ap().rearrange("(k p) c -> p k c", p=128)
                         for t in y0loc_chunks]
                cpc = SBC // NCOL  # NCOL chunks per AG chunk

                def rhs1(c):
                    part = ppool.tile([128, F, NCOL], BF16)
                    ch, off = divmod(c, cpc)
                    nc.sync.dma_start(part[:], y0l_r[ch][:, :, ts(off, NCOL)])
                    return [y0own[:, k, ts(c, NCOL)] for k in range(F)] + \
                           [part[:, k, :] for k in range(F)]

                _p_phase(pctx, tc, nc, wih1T, gbias1, gx1T, KI1, rhs1, "p1",
                         c_order=list(reversed(range(SB // NCOL))))

            # ---- S1: layer-1 scan -> y1T ----
            with ExitStack() as sctx:
                _s_phase(sctx, tc, nc, whh1T, nbias1, gx1T, 1, None, y1T,
                         ident, None)

    nc.compile()
    return nc


_PROGRAM_CACHE = {}


def _get_program():
    if "nc" not in _PROGRAM_CACHE:
        _PROGRAM_CACHE["nc"] = build_program()
    return _PROGRAM_CACHE["nc"]


def _host_inputs(inputs):
    """Build the 8 per-core input maps from the full problem inputs."""
    bf = ml_dtypes.bfloat16
    f8 = ml_dtypes.float8_e4m3
    x = np.asarray(inputs["input"], np.float32)            # (S, B, I)
    in_maps = []
    for c in range(NCORE):
        fwd = c % 2 == 0
        q = c // 2
        d = "f" if fwd else "b"
        xq = x[:, q * BQ:(q + 1) * BQ, :]
        if not fwd:
            xq = xq[::-1]
        xTv = np.ascontiguousarray(xq.transpose(2, 0, 1).reshape(I, SB))

        def wT(wname):
            return np.ascontiguousarray(np.asarray(inputs[wname], np.float32).T)

        wih0 = wT(f"Wih_{d}0")        # (I, G)
        whh0 = wT(f"Whh_{d}0")        # (H, G)
        wih1_full = wT(f"Wih_{d}1")   # (2H, G); rows = y0 features [hf | hb]
        own_sl = slice(0, H) if fwd else slice(H, 2 * H)
        par_sl = slice(H, 2 * H) if fwd else slice(0, H)
        wih1 = np.concatenate([wih1_full[own_sl], wih1_full[par_sl]], axis=0)
        whh1 = wT(f"Whh_{d}1")

        def gbias(layer):
            bih = np.asarray(inputs[f"bih_{d}{layer}"], np.float32)
            bhh = np.asarray(inputs[f"bhh_{d}{layer}"], np.float32)
            gb = np.concatenate([bih[:2 * H] + bhh[:2 * H], bih[2 * H:]])
            gb = np.ascontiguousarray(gb.reshape(M12, 128).T)  # [128, M12]
            gb[:, : 2 * F] *= WSCALE   # r,z chunks pre-scaled (see _p_phase)
            return gb

        def nbias(layer):
            bhh = np.asarray(inputs[f"bhh_{d}{layer}"], np.float32)
            nb = (bhh[2 * H:] * WSCALE).reshape(F, 128).T  # [128, F], scaled
            return np.ascontiguousarray(
                np.broadcast_to(nb[:, :, None], (128, F, BQ)).reshape(
                    128, F * BQ)).astype(bf)

        in_maps.append({
            "xT": xTv.astype(bf),
            "wih0T": wih0.astype(bf),
            "whh0T": (whh0 * WSCALE).astype(f8),
            "wih1T": wih1.astype(bf),
            "whh1T": (whh1 * WSCALE).astype(f8),
            "gbias0": gbias(0), "gbias1": gbias(1),
            "nbias0": nbias(0), "nbias1": nbias(1),
            "ident": np.eye(128, dtype=bf),
        })
    return in_maps


def kernel(**inputs) -> np.ndarray:
    nc = _get_program()
    in_maps = _host_inputs(inputs)
    trace = bool(int(os.environ.get("BIGRU_TRACE", "0")))
    kw = {}
    if trace and os.environ.get("BIGRU_TRACE_DIR"):
        kw["tmpdir"] = os.environ["BIGRU_TRACE_DIR"]
    res = run_bass_kernel_spmd(nc, in_maps, list(range(NCORE)), trace=trace, **kw)
    if trace and res.exec_time_ns is not None:
        print(f"HW exec time: {res.exec_time_ns} ns")
        _PROGRAM_CACHE["exec_time_ns"] = res.exec_time_ns
        _PROGRAM_CACHE["profile_json"] = res.profile_json

    out = np.empty((S, B, 2 * H), np.float32)
    for c in range(NCORE):
        fwd = c % 2 == 0
        q = c // 2
        y = np.asarray(res.results[c]["y1T"], dtype=np.float32)
        y = y.reshape(H, S, BQ).transpose(1, 2, 0)  # (S, BQ, H)
        if not fwd:
            y = y[::-1]
        out[:, q * BQ:(q + 1) * BQ, (0 if fwd else H):(H if fwd else 2 * H)] = y
    return out
